# revision 9
# baseline (speedup 1.0000x reference)
"""ArcNegFace loss kernel for 8 TRN2 NeuronCores.

Strategy (classifier/model parallel, Partial-FC style):
  - Shard the class dim C=100000 across 8 cores (12500 classes each,
    padded to 12800 so every core runs 25 chunks of 512).
  - Host ships per core: the transposed bf16 weight shard wtb [128, 12800]
    (layout prep: [D, C_shard]), the per-class inverse row norms broadcast
    to sbb [128, 12800] bf16, the replicated feats [512, 128], and the
    label-gathered weight rows wlb [512, 128] (per-shard label handling is
    host-side; each core recomputes a_lb redundantly -> no collectives).
  - Device per core: wnt = wtb * sbb (normalize); feats normalized *64 and
    PE-transposed into lhsT [128, 128] x4; main loop: matmul -> psum holds
    64*cos; ScalarE Square(psum/64 - a) -> d2; ScalarE Exp(-d2/2 + ln 76.8)
    -> r64 = 64*1.2*exp(-(cos-a)^2/2); DVE (psum+64)*r64 -> o; o-64 -> out.
    out = 64*((1-onehot)*(r*cos + r - 1) + onehot*a) with the onehot
    positions (one per row) patched on the host from a_lb.
"""

import math
import os
import sys

import numpy as np

for _p in ("/opt/trn_rl_repo",):
    if _p not in sys.path and os.path.isdir(_p):
        sys.path.insert(0, _p)

import ml_dtypes  # noqa: E402

B, D, C, NCORES = 512, 128, 100000, 8
CS = C // NCORES  # 12500
CSP = 12800  # padded per-core class count: 25 chunks of 512
WSUP = 2048  # epilogue supertile free dim (4 PSUM banks)
MARGIN = 0.5
SCALE = 64.0
ALPHA = 1.2
SIGMA = 2.0
THRESH = math.cos(math.pi - MARGIN)
MM = math.sin(math.pi - MARGIN) * MARGIN
LN_BIAS = math.log(ALPHA)  # r = exp(-(cos-a)^2/SIGMA + ln(ALPHA)); out = (64cos+64)*r - 64

_COMPILED = None


def _build_kernel():
    import concourse.bass as bass
    import concourse.tile as tile
    from concourse import bacc, mybir
    from concourse.masks import make_identity
    from contextlib import ExitStack

    F32 = mybir.dt.float32
    BF16 = mybir.dt.bfloat16
    OP = mybir.AluOpType
    ACT = mybir.ActivationFunctionType
    AX = mybir.AxisListType

    nc = bacc.Bacc(
        "TRN2",
        target_bir_lowering=False,
        debug=False,
        enable_asserts=False,
        num_devices=NCORES,
    )
    feats = nc.dram_tensor("feats", [B, D], F32, kind="ExternalInput").ap()
    wlb = nc.dram_tensor("wlb", [B, D], F32, kind="ExternalInput").ap()
    wntd = nc.dram_tensor("wnt", [D, CSP], BF16, kind="ExternalInput").ap()
    out = nc.dram_tensor("out", [B, CSP], F32, kind="ExternalOutput").ap()

    supers = [(i * WSUP, WSUP) for i in range(CSP // WSUP)]
    if CSP % WSUP:
        supers.append((CSP - CSP % WSUP, CSP % WSUP))

    with tile.TileContext(nc) as tc, ExitStack() as ctx:
        persist = ctx.enter_context(tc.tile_pool(name="persist", bufs=1))
        work = ctx.enter_context(tc.tile_pool(name="work", bufs=2))
        psum = ctx.enter_context(tc.tile_pool(name="psum", bufs=2, space="PSUM"))
        sbp = ctx.enter_context(tc.tile_pool(name="sbp", bufs=2))
        outp = ctx.enter_context(tc.tile_pool(name="outp", bufs=2))

        ident = persist.tile([128, 128], BF16, name="ident")
        make_identity(nc, ident[:])

        # ---- normalized transposed weights (per-supertile DMA chunks) ----
        wnt = persist.tile([D, CSP], BF16, name="wnt")
        for off, w in supers:
            nc.sync.dma_start(wnt[:, off:off + w], wntd[:, off:off + w])

        # ---- feats + label-row prep (4 batch tiles of 128) ----
        ex64t = []  # lhsT tiles [D, 128] bf16
        nega = []  # -a_lb per batch tile [128, 1] f32
        ln_bias = persist.tile([128, 1], F32, name="ln_bias")
        nc.vector.memset(ln_bias[:], LN_BIAS)
        for b in range(4):
            rows = slice(b * 128, (b + 1) * 128)
            fe = work.tile([128, D], F32, tag="fe")
            nc.sync.dma_start(fe[:], feats[rows, :])
            wl = work.tile([128, D], F32, tag="wl")
            nc.sync.dma_start(wl[:], wlb[rows, :])

            # 64/||f|| per row
            sqf = work.tile([128, D], F32, tag="sqf")
            ssf = work.tile([128, 1], F32, tag="ssf")
            nc.vector.tensor_tensor(sqf[:], fe[:], fe[:], op=OP.mult)
            nc.vector.tensor_reduce(ssf[:], sqf[:], axis=AX.X, op=OP.add)
            invf = work.tile([128, 1], F32, tag="invf")
            nc.vector.reciprocal(invf[:], ssf[:])
            s64 = work.tile([128, 1], F32, tag="s64")
            nc.scalar.activation(s64[:], invf[:], ACT.Sqrt, scale=4096.0)
            ex64 = work.tile([128, D], F32, tag="ex64")
            nc.vector.tensor_scalar(ex64[:], fe[:], s64[:], None, op0=OP.mult)

            # 1/||w_label|| per row
            sqw = work.tile([128, D], F32, tag="sqw")
            ssw = work.tile([128, 1], F32, tag="ssw")
            nc.vector.tensor_tensor(sqw[:], wl[:], wl[:], op=OP.mult)
            nc.vector.tensor_reduce(ssw[:], sqw[:], axis=AX.X, op=OP.add)
            invw = work.tile([128, 1], F32, tag="invw")
            nc.vector.reciprocal(invw[:], ssw[:])
            sw = work.tile([128, 1], F32, tag="sw")
            nc.scalar.activation(sw[:], invw[:], ACT.Sqrt)
            wln = work.tile([128, D], F32, tag="wln")
            nc.vector.tensor_scalar(wln[:], wl[:], sw[:], None, op0=OP.mult)

            # cl64 = 64*cos_lb
            junk = work.tile([128, D], F32, tag="junk")
            cl64 = work.tile([128, 1], F32, tag="cl64")
            nc.vector.scalar_tensor_tensor(
                junk[:], ex64[:], 1.0, wln[:], op0=OP.bypass, op1=OP.mult,
                accum_out=cl64[:],
            )

            # a_lb = where(cos > THRESH, cos*cosM - sinM*sqrt(1-cos^2), cos - MM)
            xc = work.tile([128, 1], F32, tag="xc")
            nc.vector.tensor_scalar(
                xc[:], cl64[:], 64.0, -64.0, op0=OP.min, op1=OP.max
            )
            x2 = work.tile([128, 1], F32, tag="x2")
            nc.scalar.activation(x2[:], xc[:], ACT.Square, scale=1.0 / 64.0)
            g = work.tile([128, 1], F32, tag="g")
            nc.scalar.activation(g[:], x2[:], ACT.Sqrt, bias=1.0, scale=-1.0)
            gs = work.tile([128, 1], F32, tag="gs")
            nc.vector.tensor_scalar(gs[:], g[:], math.sin(MARGIN), None, op0=OP.mult)
            v1 = work.tile([128, 1], F32, tag="v1")
            nc.vector.scalar_tensor_tensor(
                v1[:], xc[:], math.cos(MARGIN) / 64.0, gs[:],
                op0=OP.mult, op1=OP.subtract,
            )
            v2 = work.tile([128, 1], F32, tag="v2")
            nc.vector.tensor_scalar(
                v2[:], cl64[:], 1.0 / 64.0, MM, op0=OP.mult, op1=OP.subtract
            )
            mask = work.tile([128, 1], mybir.dt.uint8, tag="mask")
            nc.vector.tensor_scalar(
                mask[:], cl64[:], THRESH * 64.0, None, op0=OP.is_gt
            )
            a = work.tile([128, 1], F32, tag="a")
            nc.vector.select(a[:], mask[:], v1[:], v2[:])
            na = persist.tile([128, 1], F32, name=f"nega{b}")
            nc.vector.tensor_scalar(na[:], a[:], -1.0, None, op0=OP.mult)
            nega.append(na)

            # lhsT = transpose(ex64 as bf16)
            exb = work.tile([128, D], BF16, tag="exb")
            nc.vector.tensor_copy(exb[:], ex64[:])
            tp = psum.tile([128, 128], BF16, tag="ps")
            nc.tensor.transpose(tp[:], exb[:], ident[:])
            ext = persist.tile([D, 128], BF16, name=f"ex64t{b}")
            nc.vector.tensor_copy(ext[:], tp[:])
            ex64t.append(ext)

        # ---- main loop ----
        for b in range(4):
            rows = slice(b * 128, (b + 1) * 128)
            for off, w in supers:
                ps = psum.tile([128, WSUP], F32, tag="ps")
                for j in range(w // 512):
                    csl = slice(off + j * 512, off + (j + 1) * 512)
                    nc.tensor.matmul(
                        ps[:, j * 512:(j + 1) * 512],
                        ex64t[b][:],
                        wnt[:, csl],
                        start=True,
                        stop=True,
                    )
                psv = ps[:, 0:w]
                d2 = sbp.tile([128, WSUP], F32, tag="d2")
                nc.scalar.activation(
                    d2[:, 0:w], psv, ACT.Square, bias=nega[b][:], scale=1.0 / 64.0
                )
                r64 = sbp.tile([128, WSUP], F32, tag="r64")
                nc.scalar.activation(
                    r64[:, 0:w], d2[:, 0:w], ACT.Exp,
                    bias=ln_bias[:], scale=-1.0 / SIGMA,
                )
                o = outp.tile([128, WSUP], F32, tag="o")
                nc.vector.scalar_tensor_tensor(
                    o[:, 0:w], psv, 64.0, r64[:, 0:w], op0=OP.add, op1=OP.mult
                )
                outf = outp.tile([128, WSUP], F32, tag="outf")
                nc.gpsimd.tensor_scalar(
                    outf[:, 0:w], o[:, 0:w], -64.0, None, op0=OP.add
                )
                nc.sync.dma_start(out[rows, off:off + w], outf[:, 0:w])

    nc.compile()
    return nc


def _get_compiled():
    global _COMPILED
    if _COMPILED is None:
        _COMPILED = _build_kernel()
    return _COMPILED


def _host_prep(feats, labels, weight):
    """Shard + layout inputs for the 8 cores."""
    bf16 = ml_dtypes.bfloat16
    feats = np.ascontiguousarray(feats, dtype=np.float32)
    weight = np.ascontiguousarray(weight, dtype=np.float32)
    labels_i = np.asarray(labels).astype(np.int64)

    inv_norm = (1.0 / np.sqrt((weight.astype(np.float32) ** 2).sum(axis=1))).astype(
        np.float32
    )  # [C]
    wlb = np.ascontiguousarray(weight[labels_i])  # [B, D] f32

    in_maps = []
    for m in range(NCORES):
        sl = slice(m * CS, (m + 1) * CS)
        wpad = np.ones((CSP, D), dtype=np.float32)
        wpad[:CS] = weight[sl]
        s_m = np.full((CSP,), 1.0 / math.sqrt(D), dtype=np.float32)
        s_m[:CS] = inv_norm[sl]
        wnt_m = np.ascontiguousarray((wpad * s_m[:, None]).T.astype(bf16))
        in_maps.append({"feats": feats, "wlb": wlb, "wnt": wnt_m})
    return in_maps, labels_i


def _host_alb(feats, labels_i, weight):
    """Reference-exact a_lb for the label positions (host fixup)."""
    f = feats.astype(np.float64)
    ex = f / np.linalg.norm(f, axis=1, keepdims=True)
    wl = weight[labels_i].astype(np.float64)
    ewl = wl / np.linalg.norm(wl, axis=1, keepdims=True)
    cos_lb = (ex * ewl).sum(axis=1)
    a = np.where(
        cos_lb > THRESH,
        np.cos(np.arccos(np.clip(cos_lb, -1.0, 1.0)) + MARGIN),
        cos_lb - MM,
    )
    return a.astype(np.float32)


def _install_axon_profile_hook():
    """The agent image's antenv lacks axon_hooks; recreate it so
    run_bass_kernel_spmd(trace=True) can capture NTFF profiles."""
    import types

    try:
        import antenv
    except ImportError:
        return
    if "antenv.axon_hooks" not in sys.modules:
        mod = types.ModuleType("antenv.axon_hooks")
        _h = {"hook": None}
        mod.set_axon_ntff_profile_hook = lambda h: _h.__setitem__("hook", h)
        mod.get_axon_ntff_profile_hook = lambda: _h["hook"]
        sys.modules["antenv.axon_hooks"] = mod
        antenv.axon_hooks = mod
        try:
            from trn_agent_boot.trn_boot import _ntff_profile_via_ctypes

            so = os.environ.get("PJRT_LIBRARY_PATH", "/opt/axon/libaxon_pjrt.so")
            hook = _ntff_profile_via_ctypes(so)
            if hook is not None:
                mod.set_axon_ntff_profile_hook(hook)
        except Exception as e:  # noqa: BLE001
            print("ntff hook install failed:", e)
    from concourse import bass_utils

    bass_utils.upload_artifacts = lambda tmpdir: tmpdir  # zero-egress container


def _run(feats, labels, weight, trace=False, **trace_kwargs):
    from concourse import bass_utils

    if trace:
        _install_axon_profile_hook()
    nc = _get_compiled()
    in_maps, labels_i = _host_prep(feats, labels, weight)
    res = bass_utils.run_bass_kernel_spmd(
        nc, in_maps, core_ids=list(range(NCORES)), trace=trace, **trace_kwargs
    )
    out = np.empty((B, C), dtype=np.float32)
    for m in range(NCORES):
        out[:, m * CS:(m + 1) * CS] = res.results[m]["out"][:, :CS]
    a = _host_alb(np.asarray(feats, dtype=np.float32), labels_i,
                  np.asarray(weight, dtype=np.float32))
    out[np.arange(B), labels_i] = SCALE * a
    return out, res


def kernel(feats, labels, weight):
    out, _ = _run(feats, labels, weight, trace=False)
    return out


# revision 11
# speedup vs baseline: 4.0264x; 4.0264x over previous
"""ArcNegFace loss kernel for 8 TRN2 NeuronCores.

Strategy (classifier/model parallel, Partial-FC style):
  - Shard the class dim C=100000 across 8 cores (12500 classes each,
    padded to 12800 so every core runs 25 chunks of 512).
  - Host ships per core: the transposed bf16 weight shard wtb [128, 12800]
    (layout prep: [D, C_shard]), the per-class inverse row norms broadcast
    to sbb [128, 12800] bf16, the replicated feats [512, 128], and the
    label-gathered weight rows wlb [512, 128] (per-shard label handling is
    host-side; each core recomputes a_lb redundantly -> no collectives).
  - Device per core: wnt = wtb * sbb (normalize); feats normalized *64 and
    PE-transposed into lhsT [128, 128] x4; main loop: matmul -> psum holds
    64*cos; ScalarE Square(psum/64 - a) -> d2; ScalarE Exp(-d2/2 + ln 76.8)
    -> r64 = 64*1.2*exp(-(cos-a)^2/2); DVE (psum+64)*r64 -> o; o-64 -> out.
    out = 64*((1-onehot)*(r*cos + r - 1) + onehot*a) with the onehot
    positions (one per row) patched on the host from a_lb.
"""

import math
import os
import sys

import numpy as np

for _p in ("/opt/trn_rl_repo",):
    if _p not in sys.path and os.path.isdir(_p):
        sys.path.insert(0, _p)

import ml_dtypes  # noqa: E402

B, D, C, NCORES = 512, 128, 100000, 8
CS = C // NCORES  # 12500
CSP = 12800  # padded per-core class count: 25 chunks of 512
WSUP = 2048  # epilogue supertile free dim (4 PSUM banks)
MARGIN = 0.5
SCALE = 64.0
ALPHA = 1.2
SIGMA = 2.0
THRESH = math.cos(math.pi - MARGIN)
MM = math.sin(math.pi - MARGIN) * MARGIN
LN_BIAS = math.log(ALPHA)  # r = exp(-(cos-a)^2/SIGMA + ln(ALPHA)); out = (64cos+64)*r - 64

_COMPILED = None


def _build_kernel():
    import concourse.bass as bass
    import concourse.tile as tile
    from concourse import bacc, mybir
    from concourse.masks import make_identity
    from contextlib import ExitStack

    F32 = mybir.dt.float32
    BF16 = mybir.dt.bfloat16
    OP = mybir.AluOpType
    ACT = mybir.ActivationFunctionType
    AX = mybir.AxisListType

    nc = bacc.Bacc(
        "TRN2",
        target_bir_lowering=False,
        debug=False,
        enable_asserts=False,
        num_devices=NCORES,
    )
    feats = nc.dram_tensor("feats", [B, D], F32, kind="ExternalInput").ap()
    wlb = nc.dram_tensor("wlb", [B, D], F32, kind="ExternalInput").ap()
    wntd = nc.dram_tensor("wnt", [D, CSP], BF16, kind="ExternalInput").ap()
    out = nc.dram_tensor("out", [B, CSP], F32, kind="ExternalOutput").ap()

    supers = [(i * WSUP, WSUP) for i in range(CSP // WSUP)]
    if CSP % WSUP:
        supers.append((CSP - CSP % WSUP, CSP % WSUP))

    with tile.TileContext(nc) as tc, ExitStack() as ctx:
        persist = ctx.enter_context(tc.tile_pool(name="persist", bufs=1))
        work = ctx.enter_context(tc.tile_pool(name="work", bufs=2))
        psum = ctx.enter_context(tc.tile_pool(name="psum", bufs=2, space="PSUM"))
        sbp = ctx.enter_context(tc.tile_pool(name="sbp", bufs=3))
        outp = ctx.enter_context(tc.tile_pool(name="outp", bufs=3))

        ident = persist.tile([128, 128], BF16, name="ident")
        make_identity(nc, ident[:])

        # ---- normalized transposed weights (per-supertile DMA chunks) ----
        wnt = persist.tile([D, CSP], BF16, name="wnt")
        for off, w in supers:
            nc.sync.dma_start(wnt[:, off:off + w], wntd[:, off:off + w])

        # ---- feats + label-row prep (4 batch tiles of 128) ----
        ex64t = []  # lhsT tiles [D, 128] bf16
        nega = []  # -a_lb per batch tile [128, 1] f32
        ln_bias = persist.tile([128, 1], F32, name="ln_bias")
        nc.vector.memset(ln_bias[:], LN_BIAS)
        for b in range(4):
            rows = slice(b * 128, (b + 1) * 128)
            fe = work.tile([128, D], F32, tag="fe")
            nc.sync.dma_start(fe[:], feats[rows, :])
            wl = work.tile([128, D], F32, tag="wl")
            nc.sync.dma_start(wl[:], wlb[rows, :])

            # 64/||f|| per row
            sqf = work.tile([128, D], F32, tag="sqf")
            ssf = work.tile([128, 1], F32, tag="ssf")
            nc.vector.tensor_tensor(sqf[:], fe[:], fe[:], op=OP.mult)
            nc.vector.tensor_reduce(ssf[:], sqf[:], axis=AX.X, op=OP.add)
            invf = work.tile([128, 1], F32, tag="invf")
            nc.vector.reciprocal(invf[:], ssf[:])
            s64 = work.tile([128, 1], F32, tag="s64")
            nc.scalar.activation(s64[:], invf[:], ACT.Sqrt, scale=4096.0)
            ex64 = work.tile([128, D], F32, tag="ex64")
            nc.vector.tensor_scalar(ex64[:], fe[:], s64[:], None, op0=OP.mult)

            # 1/||w_label|| per row
            sqw = work.tile([128, D], F32, tag="sqw")
            ssw = work.tile([128, 1], F32, tag="ssw")
            nc.vector.tensor_tensor(sqw[:], wl[:], wl[:], op=OP.mult)
            nc.vector.tensor_reduce(ssw[:], sqw[:], axis=AX.X, op=OP.add)
            invw = work.tile([128, 1], F32, tag="invw")
            nc.vector.reciprocal(invw[:], ssw[:])
            sw = work.tile([128, 1], F32, tag="sw")
            nc.scalar.activation(sw[:], invw[:], ACT.Sqrt)
            wln = work.tile([128, D], F32, tag="wln")
            nc.vector.tensor_scalar(wln[:], wl[:], sw[:], None, op0=OP.mult)

            # cl64 = 64*cos_lb
            junk = work.tile([128, D], F32, tag="junk")
            cl64 = work.tile([128, 1], F32, tag="cl64")
            nc.vector.scalar_tensor_tensor(
                junk[:], ex64[:], 1.0, wln[:], op0=OP.bypass, op1=OP.mult,
                accum_out=cl64[:],
            )

            # a_lb = where(cos > THRESH, cos*cosM - sinM*sqrt(1-cos^2), cos - MM)
            xc = work.tile([128, 1], F32, tag="xc")
            nc.vector.tensor_scalar(
                xc[:], cl64[:], 64.0, -64.0, op0=OP.min, op1=OP.max
            )
            x2 = work.tile([128, 1], F32, tag="x2")
            nc.scalar.activation(x2[:], xc[:], ACT.Square, scale=1.0 / 64.0)
            g = work.tile([128, 1], F32, tag="g")
            nc.scalar.activation(g[:], x2[:], ACT.Sqrt, bias=1.0, scale=-1.0)
            gs = work.tile([128, 1], F32, tag="gs")
            nc.vector.tensor_scalar(gs[:], g[:], math.sin(MARGIN), None, op0=OP.mult)
            v1 = work.tile([128, 1], F32, tag="v1")
            nc.vector.scalar_tensor_tensor(
                v1[:], xc[:], math.cos(MARGIN) / 64.0, gs[:],
                op0=OP.mult, op1=OP.subtract,
            )
            v2 = work.tile([128, 1], F32, tag="v2")
            nc.vector.tensor_scalar(
                v2[:], cl64[:], 1.0 / 64.0, MM, op0=OP.mult, op1=OP.subtract
            )
            mask = work.tile([128, 1], mybir.dt.uint8, tag="mask")
            nc.vector.tensor_scalar(
                mask[:], cl64[:], THRESH * 64.0, None, op0=OP.is_gt
            )
            a = work.tile([128, 1], F32, tag="a")
            nc.vector.select(a[:], mask[:], v1[:], v2[:])
            na = persist.tile([128, 1], F32, name=f"nega{b}")
            nc.vector.tensor_scalar(na[:], a[:], -1.0, None, op0=OP.mult)
            nega.append(na)

            # lhsT = transpose(ex64 as bf16)
            exb = work.tile([128, D], BF16, tag="exb")
            nc.vector.tensor_copy(exb[:], ex64[:])
            tp = psum.tile([128, 128], BF16, tag="ps")
            nc.tensor.transpose(tp[:], exb[:], ident[:])
            ext = persist.tile([D, 128], BF16, name=f"ex64t{b}")
            nc.vector.tensor_copy(ext[:], tp[:])
            ex64t.append(ext)

        # ---- main loop ----
        for b in range(4):
            rows = slice(b * 128, (b + 1) * 128)
            for off, w in supers:
                ps = psum.tile([128, WSUP], F32, tag="ps")
                for j in range(w // 512):
                    csl = slice(off + j * 512, off + (j + 1) * 512)
                    nc.tensor.matmul(
                        ps[:, j * 512:(j + 1) * 512],
                        ex64t[b][:],
                        wnt[:, csl],
                        start=True,
                        stop=True,
                    )
                psv = ps[:, 0:w]
                d2 = sbp.tile([128, WSUP], F32, tag="d2")
                nc.scalar.activation(
                    d2[:, 0:w], psv, ACT.Square, bias=nega[b][:], scale=1.0 / 64.0
                )
                r64 = sbp.tile([128, WSUP], F32, tag="r64")
                nc.scalar.activation(
                    r64[:, 0:w], d2[:, 0:w], ACT.Exp,
                    bias=ln_bias[:], scale=-1.0 / SIGMA,
                )
                o = outp.tile([128, WSUP], F32, tag="o")
                nc.vector.scalar_tensor_tensor(
                    o[:, 0:w], psv, 64.0, r64[:, 0:w], op0=OP.add, op1=OP.mult
                )
                outf = outp.tile([128, WSUP], F32, tag="outf")
                nc.vector.tensor_scalar(
                    outf[:, 0:w], o[:, 0:w], -64.0, None, op0=OP.add
                )
                nc.sync.dma_start(out[rows, off:off + w], outf[:, 0:w])

    nc.compile()
    return nc


def _get_compiled():
    global _COMPILED
    if _COMPILED is None:
        _COMPILED = _build_kernel()
    return _COMPILED


def _host_prep(feats, labels, weight):
    """Shard + layout inputs for the 8 cores."""
    bf16 = ml_dtypes.bfloat16
    feats = np.ascontiguousarray(feats, dtype=np.float32)
    weight = np.ascontiguousarray(weight, dtype=np.float32)
    labels_i = np.asarray(labels).astype(np.int64)

    inv_norm = (1.0 / np.sqrt((weight.astype(np.float32) ** 2).sum(axis=1))).astype(
        np.float32
    )  # [C]
    wlb = np.ascontiguousarray(weight[labels_i])  # [B, D] f32

    in_maps = []
    for m in range(NCORES):
        sl = slice(m * CS, (m + 1) * CS)
        wpad = np.ones((CSP, D), dtype=np.float32)
        wpad[:CS] = weight[sl]
        s_m = np.full((CSP,), 1.0 / math.sqrt(D), dtype=np.float32)
        s_m[:CS] = inv_norm[sl]
        wnt_m = np.ascontiguousarray((wpad * s_m[:, None]).T.astype(bf16))
        in_maps.append({"feats": feats, "wlb": wlb, "wnt": wnt_m})
    return in_maps, labels_i


def _host_alb(feats, labels_i, weight):
    """Reference-exact a_lb for the label positions (host fixup)."""
    f = feats.astype(np.float64)
    ex = f / np.linalg.norm(f, axis=1, keepdims=True)
    wl = weight[labels_i].astype(np.float64)
    ewl = wl / np.linalg.norm(wl, axis=1, keepdims=True)
    cos_lb = (ex * ewl).sum(axis=1)
    a = np.where(
        cos_lb > THRESH,
        np.cos(np.arccos(np.clip(cos_lb, -1.0, 1.0)) + MARGIN),
        cos_lb - MM,
    )
    return a.astype(np.float32)


def _install_axon_profile_hook():
    """The agent image's antenv lacks axon_hooks; recreate it so
    run_bass_kernel_spmd(trace=True) can capture NTFF profiles."""
    import types

    try:
        import antenv
    except ImportError:
        return
    if "antenv.axon_hooks" not in sys.modules:
        mod = types.ModuleType("antenv.axon_hooks")
        _h = {"hook": None}
        mod.set_axon_ntff_profile_hook = lambda h: _h.__setitem__("hook", h)
        mod.get_axon_ntff_profile_hook = lambda: _h["hook"]
        sys.modules["antenv.axon_hooks"] = mod
        antenv.axon_hooks = mod
        try:
            from trn_agent_boot.trn_boot import _ntff_profile_via_ctypes

            so = os.environ.get("PJRT_LIBRARY_PATH", "/opt/axon/libaxon_pjrt.so")
            hook = _ntff_profile_via_ctypes(so)
            if hook is not None:
                mod.set_axon_ntff_profile_hook(hook)
        except Exception as e:  # noqa: BLE001
            print("ntff hook install failed:", e)
    from concourse import bass_utils

    bass_utils.upload_artifacts = lambda tmpdir: tmpdir  # zero-egress container


def _run(feats, labels, weight, trace=False, **trace_kwargs):
    from concourse import bass_utils

    if trace:
        _install_axon_profile_hook()
    nc = _get_compiled()
    in_maps, labels_i = _host_prep(feats, labels, weight)
    res = bass_utils.run_bass_kernel_spmd(
        nc, in_maps, core_ids=list(range(NCORES)), trace=trace, **trace_kwargs
    )
    out = np.empty((B, C), dtype=np.float32)
    for m in range(NCORES):
        out[:, m * CS:(m + 1) * CS] = res.results[m]["out"][:, :CS]
    a = _host_alb(np.asarray(feats, dtype=np.float32), labels_i,
                  np.asarray(weight, dtype=np.float32))
    out[np.arange(B), labels_i] = SCALE * a
    return out, res


def kernel(feats, labels, weight):
    out, _ = _run(feats, labels, weight, trace=False)
    return out


# revision 15
# speedup vs baseline: 4.7202x; 1.1723x over previous
"""ArcNegFace loss kernel for 8 TRN2 NeuronCores.

Strategy (classifier/model parallel, Partial-FC style):
  - Shard the class dim C=100000 across 8 cores (12500 classes each,
    padded to 12800 so every core runs 25 chunks of 512).
  - Host ships per core: the transposed bf16 weight shard wtb [128, 12800]
    (layout prep: [D, C_shard]), the per-class inverse row norms broadcast
    to sbb [128, 12800] bf16, the replicated feats [512, 128], and the
    label-gathered weight rows wlb [512, 128] (per-shard label handling is
    host-side; each core recomputes a_lb redundantly -> no collectives).
  - Device per core: wnt = wtb * sbb (normalize); feats normalized *64 and
    PE-transposed into lhsT [128, 128] x4; main loop: matmul -> psum holds
    64*cos; ScalarE Square(psum/64 - a) -> d2; ScalarE Exp(-d2/2 + ln 76.8)
    -> r64 = 64*1.2*exp(-(cos-a)^2/2); DVE (psum+64)*r64 -> o; o-64 -> out.
    out = 64*((1-onehot)*(r*cos + r - 1) + onehot*a) with the onehot
    positions (one per row) patched on the host from a_lb.
"""

import math
import os
import sys

import numpy as np

for _p in ("/opt/trn_rl_repo",):
    if _p not in sys.path and os.path.isdir(_p):
        sys.path.insert(0, _p)

import ml_dtypes  # noqa: E402

B, D, C, NCORES = 512, 128, 100000, 8
CS = C // NCORES  # 12500
CSP = 12800  # padded per-core class count: 25 chunks of 512
WSUP = 2048  # epilogue supertile free dim (4 PSUM banks)
MARGIN = 0.5
SCALE = 64.0
ALPHA = 1.2
SIGMA = 2.0
THRESH = math.cos(math.pi - MARGIN)
MM = math.sin(math.pi - MARGIN) * MARGIN
LN_BIAS = math.log(ALPHA)  # r = exp(-(cos-a)^2/SIGMA + ln(ALPHA)); out = (64cos+64)*r - 64

_COMPILED = None


def _build_kernel():
    import concourse.bass as bass
    import concourse.tile as tile
    from concourse import bacc, mybir
    from concourse.masks import make_identity
    from contextlib import ExitStack

    F32 = mybir.dt.float32
    BF16 = mybir.dt.bfloat16
    OP = mybir.AluOpType
    ACT = mybir.ActivationFunctionType
    AX = mybir.AxisListType

    nc = bacc.Bacc(
        "TRN2",
        target_bir_lowering=False,
        debug=False,
        enable_asserts=False,
        num_devices=NCORES,
    )
    feats = nc.dram_tensor("feats", [B, D], F32, kind="ExternalInput").ap()
    wlb = nc.dram_tensor("wlb", [B, D], F32, kind="ExternalInput").ap()
    wntd = nc.dram_tensor("wnt", [D, CSP], BF16, kind="ExternalInput").ap()
    out = nc.dram_tensor("out", [B, CSP], BF16, kind="ExternalOutput").ap()

    supers = [(i * WSUP, WSUP) for i in range(CSP // WSUP)]
    if CSP % WSUP:
        # small tail first: cheapest tile primes the pipeline
        supers = [(CSP - CSP % WSUP, CSP % WSUP)] + supers

    with tile.TileContext(nc) as tc, ExitStack() as ctx:
        persist = ctx.enter_context(tc.tile_pool(name="persist", bufs=1))
        work = ctx.enter_context(tc.tile_pool(name="work", bufs=2))
        psum = ctx.enter_context(tc.tile_pool(name="psum", bufs=2, space="PSUM"))
        sbp = ctx.enter_context(tc.tile_pool(name="sbp", bufs=3))
        outp = ctx.enter_context(tc.tile_pool(name="outp", bufs=3))

        ident = persist.tile([128, 128], BF16, name="ident")
        make_identity(nc, ident[:])

        # ---- normalized transposed weights (per-supertile DMA chunks) ----
        wnt = persist.tile([D, CSP], BF16, name="wnt")
        for off, w in supers:
            nc.sync.dma_start(wnt[:, off:off + w], wntd[:, off:off + w])

        # ---- feats + label-row prep (4 batch tiles of 128) ----
        ex64t = []  # lhsT tiles [D, 128] bf16
        nega = []  # -a_lb per batch tile [128, 1] f32
        ln_bias = persist.tile([128, 1], F32, name="ln_bias")
        nc.vector.memset(ln_bias[:], LN_BIAS)
        for b in range(4):
            rows = slice(b * 128, (b + 1) * 128)
            fe = work.tile([128, D], F32, tag="fe")
            nc.sync.dma_start(fe[:], feats[rows, :])
            wl = work.tile([128, D], F32, tag="wl")
            nc.sync.dma_start(wl[:], wlb[rows, :])

            # 64/||f|| per row
            sqf = work.tile([128, D], F32, tag="sqf")
            ssf = work.tile([128, 1], F32, tag="ssf")
            nc.vector.tensor_tensor(sqf[:], fe[:], fe[:], op=OP.mult)
            nc.vector.tensor_reduce(ssf[:], sqf[:], axis=AX.X, op=OP.add)
            invf = work.tile([128, 1], F32, tag="invf")
            nc.vector.reciprocal(invf[:], ssf[:])
            s64 = work.tile([128, 1], F32, tag="s64")
            nc.scalar.activation(s64[:], invf[:], ACT.Sqrt, scale=4096.0)
            ex64 = work.tile([128, D], F32, tag="ex64")
            nc.vector.tensor_scalar(ex64[:], fe[:], s64[:], None, op0=OP.mult)

            # 1/||w_label|| per row
            sqw = work.tile([128, D], F32, tag="sqw")
            ssw = work.tile([128, 1], F32, tag="ssw")
            nc.vector.tensor_tensor(sqw[:], wl[:], wl[:], op=OP.mult)
            nc.vector.tensor_reduce(ssw[:], sqw[:], axis=AX.X, op=OP.add)
            invw = work.tile([128, 1], F32, tag="invw")
            nc.vector.reciprocal(invw[:], ssw[:])
            sw = work.tile([128, 1], F32, tag="sw")
            nc.scalar.activation(sw[:], invw[:], ACT.Sqrt)
            wln = work.tile([128, D], F32, tag="wln")
            nc.vector.tensor_scalar(wln[:], wl[:], sw[:], None, op0=OP.mult)

            # cl64 = 64*cos_lb
            junk = work.tile([128, D], F32, tag="junk")
            cl64 = work.tile([128, 1], F32, tag="cl64")
            nc.vector.scalar_tensor_tensor(
                junk[:], ex64[:], 1.0, wln[:], op0=OP.bypass, op1=OP.mult,
                accum_out=cl64[:],
            )

            # a_lb = where(cos > THRESH, cos*cosM - sinM*sqrt(1-cos^2), cos - MM)
            xc = work.tile([128, 1], F32, tag="xc")
            nc.vector.tensor_scalar(
                xc[:], cl64[:], 64.0, -64.0, op0=OP.min, op1=OP.max
            )
            x2 = work.tile([128, 1], F32, tag="x2")
            nc.scalar.activation(x2[:], xc[:], ACT.Square, scale=1.0 / 64.0)
            g = work.tile([128, 1], F32, tag="g")
            nc.scalar.activation(g[:], x2[:], ACT.Sqrt, bias=1.0, scale=-1.0)
            gs = work.tile([128, 1], F32, tag="gs")
            nc.vector.tensor_scalar(gs[:], g[:], math.sin(MARGIN), None, op0=OP.mult)
            v1 = work.tile([128, 1], F32, tag="v1")
            nc.vector.scalar_tensor_tensor(
                v1[:], xc[:], math.cos(MARGIN) / 64.0, gs[:],
                op0=OP.mult, op1=OP.subtract,
            )
            v2 = work.tile([128, 1], F32, tag="v2")
            nc.vector.tensor_scalar(
                v2[:], cl64[:], 1.0 / 64.0, MM, op0=OP.mult, op1=OP.subtract
            )
            mask = work.tile([128, 1], mybir.dt.uint8, tag="mask")
            nc.vector.tensor_scalar(
                mask[:], cl64[:], THRESH * 64.0, None, op0=OP.is_gt
            )
            a = work.tile([128, 1], F32, tag="a")
            nc.vector.select(a[:], mask[:], v1[:], v2[:])
            na = persist.tile([128, 1], F32, name=f"nega{b}")
            nc.vector.tensor_scalar(na[:], a[:], -1.0, None, op0=OP.mult)
            nega.append(na)

            # lhsT = transpose(ex64 as bf16)
            exb = work.tile([128, D], BF16, tag="exb")
            nc.vector.tensor_copy(exb[:], ex64[:])
            tp = psum.tile([128, 128], BF16, tag="ps")
            nc.tensor.transpose(tp[:], exb[:], ident[:])
            ext = persist.tile([D, 128], BF16, name=f"ex64t{b}")
            nc.vector.tensor_copy(ext[:], tp[:])
            ex64t.append(ext)

        # ---- main loop ----
        for b in range(4):
            rows = slice(b * 128, (b + 1) * 128)
            for off, w in supers:
                ps = psum.tile([128, WSUP], F32, tag="ps")
                for j in range(w // 512):
                    csl = slice(off + j * 512, off + (j + 1) * 512)
                    nc.tensor.matmul(
                        ps[:, j * 512:(j + 1) * 512],
                        ex64t[b][:],
                        wnt[:, csl],
                        start=True,
                        stop=True,
                    )
                psv = ps[:, 0:w]
                d2 = sbp.tile([128, WSUP], F32, tag="d2")
                nc.scalar.activation(
                    d2[:, 0:w], psv, ACT.Square, bias=nega[b][:], scale=1.0 / 64.0
                )
                r64 = sbp.tile([128, WSUP], F32, tag="r64")
                nc.scalar.activation(
                    r64[:, 0:w], d2[:, 0:w], ACT.Exp,
                    bias=ln_bias[:], scale=-1.0 / SIGMA,
                )
                o = outp.tile([128, WSUP], F32, tag="o")
                nc.vector.scalar_tensor_tensor(
                    o[:, 0:w], psv, 64.0, r64[:, 0:w], op0=OP.add, op1=OP.mult
                )
                outf = outp.tile([128, WSUP], BF16, tag="outf")
                nc.vector.tensor_scalar(
                    outf[:, 0:w], o[:, 0:w], -64.0, None, op0=OP.add
                )
                nc.sync.dma_start(out[rows, off:off + w], outf[:, 0:w])

    nc.compile()
    return nc


def _get_compiled():
    global _COMPILED
    if _COMPILED is None:
        _COMPILED = _build_kernel()
    return _COMPILED


def _host_prep(feats, labels, weight):
    """Shard + layout inputs for the 8 cores."""
    bf16 = ml_dtypes.bfloat16
    feats = np.ascontiguousarray(feats, dtype=np.float32)
    weight = np.ascontiguousarray(weight, dtype=np.float32)
    labels_i = np.asarray(labels).astype(np.int64)

    inv_norm = (1.0 / np.sqrt((weight.astype(np.float32) ** 2).sum(axis=1))).astype(
        np.float32
    )  # [C]
    wlb = np.ascontiguousarray(weight[labels_i])  # [B, D] f32

    in_maps = []
    for m in range(NCORES):
        sl = slice(m * CS, (m + 1) * CS)
        wpad = np.ones((CSP, D), dtype=np.float32)
        wpad[:CS] = weight[sl]
        s_m = np.full((CSP,), 1.0 / math.sqrt(D), dtype=np.float32)
        s_m[:CS] = inv_norm[sl]
        wnt_m = np.ascontiguousarray((wpad * s_m[:, None]).T.astype(bf16))
        in_maps.append({"feats": feats, "wlb": wlb, "wnt": wnt_m})
    return in_maps, labels_i


def _host_alb(feats, labels_i, weight):
    """Reference-exact a_lb for the label positions (host fixup)."""
    f = feats.astype(np.float64)
    ex = f / np.linalg.norm(f, axis=1, keepdims=True)
    wl = weight[labels_i].astype(np.float64)
    ewl = wl / np.linalg.norm(wl, axis=1, keepdims=True)
    cos_lb = (ex * ewl).sum(axis=1)
    a = np.where(
        cos_lb > THRESH,
        np.cos(np.arccos(np.clip(cos_lb, -1.0, 1.0)) + MARGIN),
        cos_lb - MM,
    )
    return a.astype(np.float32)


def _install_axon_profile_hook():
    """The agent image's antenv lacks axon_hooks; recreate it so
    run_bass_kernel_spmd(trace=True) can capture NTFF profiles."""
    import types

    try:
        import antenv
    except ImportError:
        return
    if "antenv.axon_hooks" not in sys.modules:
        mod = types.ModuleType("antenv.axon_hooks")
        _h = {"hook": None}
        mod.set_axon_ntff_profile_hook = lambda h: _h.__setitem__("hook", h)
        mod.get_axon_ntff_profile_hook = lambda: _h["hook"]
        sys.modules["antenv.axon_hooks"] = mod
        antenv.axon_hooks = mod
        try:
            from trn_agent_boot.trn_boot import _ntff_profile_via_ctypes

            so = os.environ.get("PJRT_LIBRARY_PATH", "/opt/axon/libaxon_pjrt.so")
            hook = _ntff_profile_via_ctypes(so)
            if hook is not None:
                mod.set_axon_ntff_profile_hook(hook)
        except Exception as e:  # noqa: BLE001
            print("ntff hook install failed:", e)
    from concourse import bass_utils

    bass_utils.upload_artifacts = lambda tmpdir: tmpdir  # zero-egress container


def _run(feats, labels, weight, trace=False, **trace_kwargs):
    from concourse import bass_utils

    if trace:
        _install_axon_profile_hook()
    nc = _get_compiled()
    in_maps, labels_i = _host_prep(feats, labels, weight)
    res = bass_utils.run_bass_kernel_spmd(
        nc, in_maps, core_ids=list(range(NCORES)), trace=trace, **trace_kwargs
    )
    out = np.empty((B, C), dtype=np.float32)
    for m in range(NCORES):
        shard = res.results[m]["out"]
        out[:, m * CS:(m + 1) * CS] = shard[:, :CS].astype(np.float32)
    a = _host_alb(np.asarray(feats, dtype=np.float32), labels_i,
                  np.asarray(weight, dtype=np.float32))
    out[np.arange(B), labels_i] = SCALE * a
    return out, res


def kernel(feats, labels, weight):
    out, _ = _run(feats, labels, weight, trace=False)
    return out


# revision 19
# speedup vs baseline: 4.9098x; 1.0402x over previous
"""ArcNegFace loss kernel for 8 TRN2 NeuronCores.

Strategy (classifier/model parallel, Partial-FC style):
  - Shard the class dim C=100000 across 8 cores (12500 classes each,
    padded to 12800 so every core runs 25 chunks of 512).
  - Host ships per core: the transposed bf16 weight shard wtb [128, 12800]
    (layout prep: [D, C_shard]), the per-class inverse row norms broadcast
    to sbb [128, 12800] bf16, the replicated feats [512, 128], and the
    label-gathered weight rows wlb [512, 128] (per-shard label handling is
    host-side; each core recomputes a_lb redundantly -> no collectives).
  - Device per core: wnt = wtb * sbb (normalize); feats normalized *64 and
    PE-transposed into lhsT [128, 128] x4; main loop: matmul -> psum holds
    64*cos; ScalarE Square(psum/64 - a) -> d2; ScalarE Exp(-d2/2 + ln 76.8)
    -> r64 = 64*1.2*exp(-(cos-a)^2/2); DVE (psum+64)*r64 -> o; o-64 -> out.
    out = 64*((1-onehot)*(r*cos + r - 1) + onehot*a) with the onehot
    positions (one per row) patched on the host from a_lb.
"""

import math
import os
import sys

import numpy as np

for _p in ("/opt/trn_rl_repo",):
    if _p not in sys.path and os.path.isdir(_p):
        sys.path.insert(0, _p)

import ml_dtypes  # noqa: E402

B, D, C, NCORES = 512, 128, 100000, 8
CS = C // NCORES  # 12500
CSP = 12800  # padded per-core class count: 25 chunks of 512
WSUP = 1024  # epilogue supertile free dim (2 PSUM banks, 4 slots)
MARGIN = 0.5
SCALE = 64.0
ALPHA = 1.2
SIGMA = 2.0
THRESH = math.cos(math.pi - MARGIN)
MM = math.sin(math.pi - MARGIN) * MARGIN
LN_BIAS = math.log(ALPHA)  # r = exp(-(cos-a)^2/SIGMA + ln(ALPHA)); out = (64cos+64)*r - 64

_COMPILED = None


def _build_kernel():
    import concourse.bass as bass
    import concourse.tile as tile
    from concourse import bacc, mybir
    from concourse.masks import make_identity
    from contextlib import ExitStack

    F32 = mybir.dt.float32
    BF16 = mybir.dt.bfloat16
    OP = mybir.AluOpType
    ACT = mybir.ActivationFunctionType
    AX = mybir.AxisListType

    nc = bacc.Bacc(
        "TRN2",
        target_bir_lowering=False,
        debug=False,
        enable_asserts=False,
        num_devices=NCORES,
    )
    feats = nc.dram_tensor("feats", [B, D], F32, kind="ExternalInput").ap()
    wlb = nc.dram_tensor("wlb", [B, D], F32, kind="ExternalInput").ap()
    wntd = nc.dram_tensor("wnt", [D, CSP], BF16, kind="ExternalInput").ap()
    out = nc.dram_tensor("out", [B, CSP], BF16, kind="ExternalOutput").ap()

    supers = [(i * WSUP, WSUP) for i in range(CSP // WSUP)]
    if CSP % WSUP:
        # small tail first: cheapest tile primes the pipeline
        supers = [(CSP - CSP % WSUP, CSP % WSUP)] + supers

    with tile.TileContext(nc) as tc, ExitStack() as ctx:
        persist = ctx.enter_context(tc.tile_pool(name="persist", bufs=1))
        work = ctx.enter_context(tc.tile_pool(name="work", bufs=2))
        psum = ctx.enter_context(tc.tile_pool(name="psum", bufs=4, space="PSUM"))
        sbp = ctx.enter_context(tc.tile_pool(name="sbp", bufs=4))
        outp = ctx.enter_context(tc.tile_pool(name="outp", bufs=4))

        ident = persist.tile([128, 128], BF16, name="ident")
        make_identity(nc, ident[:])

        # ---- feats + label rows first (small DMAs ahead of the weight DMA) ----
        feat_tiles = []
        wlb_tiles = []
        for b in range(4):
            rows = slice(b * 128, (b + 1) * 128)
            fe = persist.tile([128, D], F32, tag=f"fe{b}", name=f"fe{b}")
            nc.sync.dma_start(fe[:], feats[rows, :])
            wl = persist.tile([128, D], F32, tag=f"wl{b}", name=f"wl{b}")
            nc.sync.dma_start(wl[:], wlb[rows, :])
            feat_tiles.append(fe)
            wlb_tiles.append(wl)

        # ---- normalized transposed weights (per-supertile DMA chunks) ----
        wnt = persist.tile([D, CSP], BF16, name="wnt")
        for off, w in supers:
            nc.sync.dma_start(wnt[:, off:off + w], wntd[:, off:off + w])

        # ---- feats + label-row prep (4 batch tiles of 128) ----
        ex64t = []  # lhsT tiles [D, 128] bf16
        nega = []  # -a_lb per batch tile [128, 1] f32
        ln_bias = persist.tile([128, 1], F32, name="ln_bias")
        nc.vector.memset(ln_bias[:], LN_BIAS)
        for b in range(4):
            fe = feat_tiles[b]
            wl = wlb_tiles[b]

            # 64/||f|| per row
            sqf = work.tile([128, D], F32, tag="sqf")
            ssf = work.tile([128, 1], F32, tag="ssf")
            nc.vector.tensor_tensor(sqf[:], fe[:], fe[:], op=OP.mult)
            nc.vector.tensor_reduce(ssf[:], sqf[:], axis=AX.X, op=OP.add)
            invf = work.tile([128, 1], F32, tag="invf")
            nc.vector.reciprocal(invf[:], ssf[:])
            s64 = work.tile([128, 1], F32, tag="s64")
            nc.scalar.activation(s64[:], invf[:], ACT.Sqrt, scale=4096.0)
            ex64 = work.tile([128, D], F32, tag="ex64")
            nc.vector.tensor_scalar(ex64[:], fe[:], s64[:], None, op0=OP.mult)

            # 1/||w_label|| per row
            sqw = work.tile([128, D], F32, tag="sqw")
            ssw = work.tile([128, 1], F32, tag="ssw")
            nc.vector.tensor_tensor(sqw[:], wl[:], wl[:], op=OP.mult)
            nc.vector.tensor_reduce(ssw[:], sqw[:], axis=AX.X, op=OP.add)
            invw = work.tile([128, 1], F32, tag="invw")
            nc.vector.reciprocal(invw[:], ssw[:])
            sw = work.tile([128, 1], F32, tag="sw")
            nc.scalar.activation(sw[:], invw[:], ACT.Sqrt)
            wln = work.tile([128, D], F32, tag="wln")
            nc.vector.tensor_scalar(wln[:], wl[:], sw[:], None, op0=OP.mult)

            # cl64 = 64*cos_lb
            junk = work.tile([128, D], F32, tag="junk")
            cl64 = work.tile([128, 1], F32, tag="cl64")
            nc.vector.scalar_tensor_tensor(
                junk[:], ex64[:], 1.0, wln[:], op0=OP.bypass, op1=OP.mult,
                accum_out=cl64[:],
            )

            # a_lb = where(cos > THRESH, cos*cosM - sinM*sqrt(1-cos^2), cos - MM)
            xc = work.tile([128, 1], F32, tag="xc")
            nc.vector.tensor_scalar(
                xc[:], cl64[:], 64.0, -64.0, op0=OP.min, op1=OP.max
            )
            x2 = work.tile([128, 1], F32, tag="x2")
            nc.scalar.activation(x2[:], xc[:], ACT.Square, scale=1.0 / 64.0)
            g = work.tile([128, 1], F32, tag="g")
            nc.scalar.activation(g[:], x2[:], ACT.Sqrt, bias=1.0, scale=-1.0)
            gs = work.tile([128, 1], F32, tag="gs")
            nc.vector.tensor_scalar(gs[:], g[:], math.sin(MARGIN), None, op0=OP.mult)
            v1 = work.tile([128, 1], F32, tag="v1")
            nc.vector.scalar_tensor_tensor(
                v1[:], xc[:], math.cos(MARGIN) / 64.0, gs[:],
                op0=OP.mult, op1=OP.subtract,
            )
            v2 = work.tile([128, 1], F32, tag="v2")
            nc.vector.tensor_scalar(
                v2[:], cl64[:], 1.0 / 64.0, MM, op0=OP.mult, op1=OP.subtract
            )
            mask = work.tile([128, 1], mybir.dt.uint8, tag="mask")
            nc.vector.tensor_scalar(
                mask[:], cl64[:], THRESH * 64.0, None, op0=OP.is_gt
            )
            a = work.tile([128, 1], F32, tag="a")
            nc.vector.select(a[:], mask[:], v1[:], v2[:])
            na = persist.tile([128, 1], F32, name=f"nega{b}")
            nc.vector.tensor_scalar(na[:], a[:], -1.0, None, op0=OP.mult)
            nega.append(na)

            # lhsT = transpose(ex64 as bf16)
            exb = work.tile([128, D], BF16, tag="exb")
            nc.vector.tensor_copy(exb[:], ex64[:])
            tp = psum.tile([128, 128], BF16, tag="ps")
            nc.tensor.transpose(tp[:], exb[:], ident[:])
            ext = persist.tile([D, 128], BF16, name=f"ex64t{b}")
            nc.vector.tensor_copy(ext[:], tp[:])
            ex64t.append(ext)

        # ---- main loop ----
        for b in range(4):
            rows = slice(b * 128, (b + 1) * 128)
            for off, w in supers:
                ps = psum.tile([128, WSUP], F32, tag="ps")
                for j in range(w // 512):
                    csl = slice(off + j * 512, off + (j + 1) * 512)
                    nc.tensor.matmul(
                        ps[:, j * 512:(j + 1) * 512],
                        ex64t[b][:],
                        wnt[:, csl],
                        start=True,
                        stop=True,
                    )
                psv = ps[:, 0:w]
                d2 = sbp.tile([128, WSUP], F32, tag="d2")
                nc.scalar.activation(
                    d2[:, 0:w], psv, ACT.Square, bias=nega[b][:], scale=1.0 / 64.0
                )
                r64 = sbp.tile([128, WSUP], F32, tag="r64")
                nc.scalar.activation(
                    r64[:, 0:w], d2[:, 0:w], ACT.Exp,
                    bias=ln_bias[:], scale=-1.0 / SIGMA,
                )
                o = outp.tile([128, WSUP], F32, tag="o")
                nc.vector.scalar_tensor_tensor(
                    o[:, 0:w], psv, 64.0, r64[:, 0:w], op0=OP.add, op1=OP.mult
                )
                outf = outp.tile([128, WSUP], BF16, tag="outf")
                nc.vector.tensor_scalar(
                    outf[:, 0:w], o[:, 0:w], -64.0, None, op0=OP.add
                )
                nc.sync.dma_start(out[rows, off:off + w], outf[:, 0:w])

    nc.compile()
    return nc


def _get_compiled():
    global _COMPILED
    if _COMPILED is None:
        _COMPILED = _build_kernel()
    return _COMPILED


def _host_prep(feats, labels, weight):
    """Shard + layout inputs for the 8 cores."""
    bf16 = ml_dtypes.bfloat16
    feats = np.ascontiguousarray(feats, dtype=np.float32)
    weight = np.ascontiguousarray(weight, dtype=np.float32)
    labels_i = np.asarray(labels).astype(np.int64)

    inv_norm = (1.0 / np.sqrt((weight.astype(np.float32) ** 2).sum(axis=1))).astype(
        np.float32
    )  # [C]
    wlb = np.ascontiguousarray(weight[labels_i])  # [B, D] f32

    in_maps = []
    for m in range(NCORES):
        sl = slice(m * CS, (m + 1) * CS)
        wpad = np.ones((CSP, D), dtype=np.float32)
        wpad[:CS] = weight[sl]
        s_m = np.full((CSP,), 1.0 / math.sqrt(D), dtype=np.float32)
        s_m[:CS] = inv_norm[sl]
        wnt_m = np.ascontiguousarray((wpad * s_m[:, None]).T.astype(bf16))
        in_maps.append({"feats": feats, "wlb": wlb, "wnt": wnt_m})
    return in_maps, labels_i


def _host_alb(feats, labels_i, weight):
    """Reference-exact a_lb for the label positions (host fixup)."""
    f = feats.astype(np.float64)
    ex = f / np.linalg.norm(f, axis=1, keepdims=True)
    wl = weight[labels_i].astype(np.float64)
    ewl = wl / np.linalg.norm(wl, axis=1, keepdims=True)
    cos_lb = (ex * ewl).sum(axis=1)
    a = np.where(
        cos_lb > THRESH,
        np.cos(np.arccos(np.clip(cos_lb, -1.0, 1.0)) + MARGIN),
        cos_lb - MM,
    )
    return a.astype(np.float32)


def _install_axon_profile_hook():
    """The agent image's antenv lacks axon_hooks; recreate it so
    run_bass_kernel_spmd(trace=True) can capture NTFF profiles."""
    import types

    try:
        import antenv
    except ImportError:
        return
    if "antenv.axon_hooks" not in sys.modules:
        mod = types.ModuleType("antenv.axon_hooks")
        _h = {"hook": None}
        mod.set_axon_ntff_profile_hook = lambda h: _h.__setitem__("hook", h)
        mod.get_axon_ntff_profile_hook = lambda: _h["hook"]
        sys.modules["antenv.axon_hooks"] = mod
        antenv.axon_hooks = mod
        try:
            from trn_agent_boot.trn_boot import _ntff_profile_via_ctypes

            so = os.environ.get("PJRT_LIBRARY_PATH", "/opt/axon/libaxon_pjrt.so")
            hook = _ntff_profile_via_ctypes(so)
            if hook is not None:
                mod.set_axon_ntff_profile_hook(hook)
        except Exception as e:  # noqa: BLE001
            print("ntff hook install failed:", e)
    from concourse import bass_utils

    bass_utils.upload_artifacts = lambda tmpdir: tmpdir  # zero-egress container


def _run(feats, labels, weight, trace=False, **trace_kwargs):
    from concourse import bass_utils

    if trace:
        _install_axon_profile_hook()
    nc = _get_compiled()
    in_maps, labels_i = _host_prep(feats, labels, weight)
    res = bass_utils.run_bass_kernel_spmd(
        nc, in_maps, core_ids=list(range(NCORES)), trace=trace, **trace_kwargs
    )
    out = np.empty((B, C), dtype=np.float32)
    for m in range(NCORES):
        shard = res.results[m]["out"]
        out[:, m * CS:(m + 1) * CS] = shard[:, :CS].astype(np.float32)
    a = _host_alb(np.asarray(feats, dtype=np.float32), labels_i,
                  np.asarray(weight, dtype=np.float32))
    out[np.arange(B), labels_i] = SCALE * a
    return out, res


def kernel(feats, labels, weight):
    out, _ = _run(feats, labels, weight, trace=False)
    return out
